# revision 1
# baseline (speedup 1.0000x reference)
"""Bass/Tile TRN2 kernel for GroupedQueryAttention (B=2, T=2048, D=2048,
32 Q heads / 8 KV heads, hd=64, RoPE, causal), sharded over 8 NeuronCores
by KV head (1 KV head + 4 Q heads per core; wo row-sharded, partials
summed on host)."""

import sys

for _p in ("/opt/trn_rl_repo",):
    if _p not in sys.path:
        sys.path.insert(0, _p)

import numpy as np

import concourse.bass as bass
import concourse.mybir as mybir
import concourse.tile as tile
from concourse import bacc
from concourse.bass_utils import run_bass_kernel_spmd

F32 = mybir.dt.float32
P = 128
HD = 64          # head dim
NHL = 4          # q heads per core
CH = 512         # token chunk (matmul free dim)
NCORES = 8


def build_program(B=2, T=2048, D=2048, debug_dump=False):
    """Emit the per-core SPMD program. Identical on all cores; inputs differ."""
    BT = B * T
    KT = D // P            # contraction tiles for projections
    NCH = BT // CH         # 512-token chunks over all batches
    NJ = T // CH           # tq chunks per batch
    NI = T // P            # tk tiles per batch
    NTT = BT // P          # token tiles over all batches

    nc = bacc.Bacc(None, target_bir_lowering=False, debug=False)

    xT_d = nc.dram_tensor("xT", [D, BT], F32, kind="ExternalInput")
    wq_d = nc.dram_tensor("wqT", [D, 256], F32, kind="ExternalInput")
    wkv_d = nc.dram_tensor("wkvT", [D, 128], F32, kind="ExternalInput")
    wo_d = nc.dram_tensor("woT", [256, D], F32, kind="ExternalInput")
    cs_d = nc.dram_tensor("cs", [P, T], F32, kind="ExternalInput")
    sn_d = nc.dram_tensor("sn", [P, T], F32, kind="ExternalInput")
    perm_d = nc.dram_tensor("perm", [P, P], F32, kind="ExternalInput")
    id64_d = nc.dram_tensor("id64", [HD, HD], F32, kind="ExternalInput")
    dmask_d = nc.dram_tensor("dmask", [P, P], F32, kind="ExternalInput")
    out_d = nc.dram_tensor("out", [BT, D], F32, kind="ExternalOutput")
    if debug_dump:
        qdump_d = nc.dram_tensor("qdump", [P, 2, BT], F32, kind="ExternalOutput")
        kdump_d = nc.dram_tensor("kdump", [P, B, T], F32, kind="ExternalOutput")
        vdump_d = nc.dram_tensor("vdump", [P, BT // P, HD + 1], F32,
                                 kind="ExternalOutput")
        ogdump_d = nc.dram_tensor("ogdump", [P, 2, BT], F32, kind="ExternalOutput")
        NItot = T // P
        otdump_d = nc.dram_tensor("otdump", [P, NHL, BT], F32,
                                  kind="ExternalOutput")
        pdump_d = nc.dram_tensor("pdump", [P, NCH, NItot, NHL, CH], F32,
                                 kind="ExternalOutput")

    with tile.TileContext(nc) as tc:
        with tc.tile_pool(name="persist", bufs=1) as persist:
            wq_sb = persist.tile([P, KT, 256], F32, tag="wq")
            wkv_sb = persist.tile([P, KT, 128], F32, tag="wkv")
            wo_sb = persist.tile([P, 2, D], F32, tag="wo")
            cs_sb = persist.tile([P, T], F32, tag="cs")
            sn_sb = persist.tile([P, T], F32, tag="sn")
            perm_sb = persist.tile([P, P], F32, tag="perm")
            id64_sb = persist.tile([HD, HD], F32, tag="id64")
            dmask_sb = persist.tile([P, P], F32, tag="dmask")
            ones_sb = persist.tile([P, HD], F32, tag="ones")
            q_sb = persist.tile([P, 2, BT], F32, tag="qcache")
            k_sb = persist.tile([P, B, T], F32, tag="kcache")
            v_sb = persist.tile([P, NTT, HD + 1], F32, tag="vcache")

            nc.sync.dma_start(wq_sb[:], wq_d[:].rearrange("(ko p) m -> p ko m", p=P))
            nc.sync.dma_start(wkv_sb[:], wkv_d[:].rearrange("(ko p) m -> p ko m", p=P))
            nc.sync.dma_start(wo_sb[:], wo_d[:].rearrange("(fo p) n -> p fo n", p=P))
            nc.sync.dma_start(cs_sb[:], cs_d[:])
            nc.sync.dma_start(sn_sb[:], sn_d[:])
            nc.sync.dma_start(perm_sb[:], perm_d[:])
            nc.sync.dma_start(id64_sb[:], id64_d[:])
            nc.sync.dma_start(dmask_sb[:], dmask_d[:])
            nc.vector.memset(v_sb[:, :, HD:HD + 1], 1.0)
            nc.vector.memset(ones_sb[:], 1.0)

            # ---- projections + RoPE (q,k hd-major; v token-major + ones col)
            with (
                tc.tile_pool(name="pa", bufs=5, space="PSUM") as pa,
                tc.tile_pool(name="pb", bufs=2, space="PSUM") as pb,
                tc.tile_pool(name="ptr", bufs=1, space="PSUM") as ptr,
                tc.tile_pool(name="xk", bufs=4) as xkp,
                tc.tile_pool(name="rtmp", bufs=2) as rtmp,
            ):
                for ch in range(NCH):
                    b = ch // NJ
                    tcol = ch * CH               # global token col
                    kcol = CH * (ch % NJ)        # within-batch token col
                    kvp = pa.tile([P, CH], F32, tag="pacc")
                    q0p = pa.tile([P, CH], F32, tag="pacc")
                    q1p = pa.tile([P, CH], F32, tag="pacc")
                    for k in range(KT):
                        xk = xkp.tile([P, CH], F32, tag="xk")
                        nc.sync.dma_start(
                            xk[:], xT_d[k * P:(k + 1) * P, tcol:tcol + CH])
                        st = (k == 0)
                        sp = (k == KT - 1)
                        nc.tensor.matmul(kvp[:], wkv_sb[:, k, :], xk[:],
                                         start=st, stop=sp)
                        nc.tensor.matmul(q0p[:], wq_sb[:, k, 0:P], xk[:],
                                         start=st, stop=sp)
                        nc.tensor.matmul(q1p[:], wq_sb[:, k, P:256], xk[:],
                                         start=st, stop=sp)
                    csl = cs_sb[:, kcol:kcol + CH]
                    snl = sn_sb[:, kcol:kcol + CH]
                    # q RoPE: rope(q) = q*cos + (P.T@q)*sin
                    for ht, qp in ((0, q0p), (1, q1p)):
                        qs = rtmp.tile([P, CH], F32, tag="ropea")
                        nc.scalar.copy(qs[:], qp[:])
                        qsw = pb.tile([P, CH], F32, tag="pswap")
                        nc.tensor.matmul(qsw[:], perm_sb[:], qs[:],
                                         start=True, stop=True)
                        dst = q_sb[:, ht, tcol:tcol + CH]
                        nc.vector.tensor_mul(dst, qs[:], csl)
                        t2 = rtmp.tile([P, CH], F32, tag="ropeb")
                        nc.vector.tensor_mul(t2[:], qsw[:], snl)
                        nc.vector.tensor_add(dst, dst, t2[:])
                    # k RoPE (rows 0:64 of kv psum), then duplicate to 64:128
                    ks = rtmp.tile([HD, CH], F32, tag="ropek")
                    nc.scalar.copy(ks[:], kvp[0:HD, :])
                    ksw_full = pb.tile([P, CH], F32, tag="pswap")
                    ksw = ksw_full[0:HD, :]
                    nc.tensor.matmul(ksw, perm_sb[0:HD, 0:HD], ks[:],
                                     start=True, stop=True)
                    kdst = k_sb[0:HD, b, kcol:kcol + CH]
                    nc.vector.tensor_mul(kdst, ks[:], cs_sb[0:HD, kcol:kcol + CH])
                    t2 = rtmp.tile([HD, CH], F32, tag="ropekb")
                    nc.vector.tensor_mul(t2[:], ksw, sn_sb[0:HD, kcol:kcol + CH])
                    nc.vector.tensor_add(kdst, kdst, t2[:])
                    nc.vector.tensor_copy(k_sb[HD:P, b, kcol:kcol + CH], kdst)
                    # v: copy rows 64:128, transpose 128-tok tiles to token-major
                    vs = rtmp.tile([HD, CH], F32, tag="ropev")
                    nc.scalar.copy(vs[:], kvp[HD:P, :])
                    for tt in range(CH // P):
                        vtp = ptr.tile([P, HD], F32, tag="ptr")
                        nc.tensor.transpose(vtp[:], vs[:, tt * P:(tt + 1) * P],
                                            id64_sb[:])
                        nc.vector.tensor_copy(
                            v_sb[:, ch * (CH // P) + tt, 0:HD], vtp[:])

            # ---- attention + inline output projection
            with (
                tc.tile_pool(name="po", bufs=4, space="PSUM") as po,
                tc.tile_pool(name="ps", bufs=2, space="PSUM") as ps,
                tc.tile_pool(name="pout", bufs=2, space="PSUM") as pout,
                tc.tile_pool(name="pp", bufs=4) as pp,
                tc.tile_pool(name="att", bufs=2) as att,
                tc.tile_pool(name="otp", bufs=4) as otp,
            ):
                for b in range(B):
                    for j in range(NJ):
                        ch = b * NJ + j
                        tcol = ch * CH
                        imax = (CH // P) * j + (CH // P) - 1
                        ot_acc = [po.tile([HD + 1, CH], F32, tag="po",
                                          name=f"po_{ch}_{h}")
                                  for h in range(NHL)]
                        for i in range(imax + 1):
                            c0 = max(0, P * i - CH * j)
                            for h in range(NHL):
                                hb = HD * (h % 2)
                                ht = h // 2
                                sp = ps.tile([P, CH], F32, tag="ps")
                                nc.tensor.matmul(
                                    sp[:, c0:CH],
                                    k_sb[hb:hb + HD, b, P * i:P * (i + 1)],
                                    q_sb[hb:hb + HD, ht, tcol + c0:tcol + CH],
                                    start=True, stop=True)
                                if P * i >= CH * j:  # diagonal tile: causal mask
                                    nc.vector.tensor_add(
                                        sp[:, c0:c0 + P], sp[:, c0:c0 + P],
                                        dmask_sb[:])
                                pt = pp.tile([P, CH], F32, tag="pt")
                                nc.scalar.activation(
                                    pt[:, c0:CH], sp[:, c0:CH],
                                    mybir.ActivationFunctionType.Exp,
                                    scale=0.125)
                                if debug_dump:
                                    nc.sync.dma_start(
                                        pdump_d[:, ch, i, h, c0:CH],
                                        pt[:, c0:CH])
                                nc.tensor.matmul(
                                    ot_acc[h][:, c0:CH],
                                    v_sb[:, b * NI + i, :],
                                    pt[:, c0:CH],
                                    start=(i == 0), stop=(i == imax),
                                    skip_group_check=True)
                        og_c = att.tile([P, 2, CH], F32, tag="og")
                        if debug_dump:
                            pass
                        for h in range(NHL):
                            hb = HD * (h % 2)
                            ht = h // 2
                            ot = otp.tile([HD + 1, CH], F32, tag="ot")
                            nc.scalar.copy(ot[:], ot_acc[h][:])
                            if debug_dump:
                                nc.sync.dma_start(
                                    otdump_d[0:HD + 1, h, tcol:tcol + CH], ot[:])
                            lr_ps = ps.tile([P, CH], F32, tag="ps")
                            nc.tensor.matmul(lr_ps[0:HD, :],
                                             ones_sb[HD:HD + 1, :],
                                             ot[HD:HD + 1, :],
                                             start=True, stop=True)
                            lr = otp.tile([HD, CH], F32, tag="lr")
                            nc.vector.reciprocal(lr[:], lr_ps[0:HD, :])
                            nc.vector.tensor_mul(
                                og_c[hb:hb + HD, ht, :], ot[0:HD, :], lr[:])
                        if debug_dump:
                            nc.sync.dma_start(
                                ogdump_d[:, :, tcol:tcol + CH], og_c[:])
                        for tt in range(CH // P):
                            for dc in range(D // CH):
                                op = pout.tile([P, CH], F32, tag="pout")
                                for ft in range(2):
                                    nc.tensor.matmul(
                                        op[:],
                                        og_c[:, ft, tt * P:(tt + 1) * P],
                                        wo_sb[:, ft, dc * CH:(dc + 1) * CH],
                                        start=(ft == 0), stop=(ft == 1))
                                ob = otp.tile([P, CH], F32, tag="ob")
                                nc.scalar.copy(ob[:], op[:])
                                nc.sync.dma_start(
                                    out_d[tcol + tt * P:tcol + (tt + 1) * P,
                                          dc * CH:(dc + 1) * CH],
                                    ob[:])
            if debug_dump:
                nc.sync.dma_start(qdump_d[:], q_sb[:])
                nc.sync.dma_start(kdump_d[:], k_sb[:])
                nc.sync.dma_start(vdump_d[:], v_sb[:])
    nc.compile()
    return nc


def host_prep(x, wq, wk, wv, wo, cos, sin, core, B=2, T=2048, D=2048):
    """Per-core input map. Core c owns KV head c and Q heads 4c..4c+3."""
    BT = B * T
    xT = np.ascontiguousarray(x.reshape(BT, D).T)
    wqT = np.ascontiguousarray(wq[256 * core:256 * (core + 1)].T)
    wkvT = np.ascontiguousarray(
        np.concatenate([wk[HD * core:HD * (core + 1)],
                        wv[HD * core:HD * (core + 1)]], axis=0).T)
    woT = np.ascontiguousarray(wo[:, 256 * core:256 * (core + 1)].T)
    idx = (np.arange(P) % HD) // 2
    cs = np.ascontiguousarray(cos[:T, idx].T.astype(np.float32))
    sn = np.ascontiguousarray(sin[:T, idx].T.astype(np.float32))
    perm = np.zeros((P, P), dtype=np.float32)
    ii = np.arange(0, P, 2)
    perm[ii, ii + 1] = 1.0
    perm[ii + 1, ii] = -1.0
    dmask = np.where(np.arange(P)[:, None] <= np.arange(P)[None, :],
                     0.0, -1e10).astype(np.float32)
    return {
        "xT": xT, "wqT": wqT, "wkvT": wkvT, "woT": woT,
        "cs": cs, "sn": sn, "perm": perm,
        "id64": np.eye(HD, dtype=np.float32), "dmask": dmask,
    }


_CACHE = {}


def _get_program(B, T, D):
    key = (B, T, D)
    if key not in _CACHE:
        _CACHE[key] = build_program(B, T, D)
    return _CACHE[key]


def run_on_hw(x, wq, wk, wv, wo, cos, sin, B=2, T=2048, D=2048, trace=False, **kw):
    nc = _get_program(B, T, D)
    in_maps = [host_prep(x, wq, wk, wv, wo, cos, sin, c, B, T, D)
               for c in range(NCORES)]
    res = run_bass_kernel_spmd(nc, in_maps, list(range(NCORES)), trace=trace, **kw)
    parts = [np.asarray(r["out"], dtype=np.float64) for r in res.results]
    out = sum(parts).astype(np.float32).reshape(B, T, D)
    return out, res


def kernel(x, mask, wq, wk, wv, wo, cos, sin):
    x = np.asarray(x, dtype=np.float32)
    out, _ = run_on_hw(np.asarray(x, np.float32), np.asarray(wq, np.float32),
                       np.asarray(wk, np.float32), np.asarray(wv, np.float32),
                       np.asarray(wo, np.float32), np.asarray(cos, np.float32),
                       np.asarray(sin, np.float32))
    return out



# revision 3
# speedup vs baseline: 2.0736x; 2.0736x over previous
"""Bass/Tile TRN2 kernel for GroupedQueryAttention (B=2, T=2048, D=2048,
32 Q heads / 8 KV heads, hd=64, RoPE, causal), sharded over 8 NeuronCores
by KV head (1 KV head + 4 Q heads per core; wo row-sharded, partials
summed on host). bf16 datapath (fp32 PSUM accumulation)."""

import sys

for _p in ("/opt/trn_rl_repo",):
    if _p not in sys.path:
        sys.path.insert(0, _p)

import numpy as np
import ml_dtypes

import concourse.bass as bass
import concourse.mybir as mybir
import concourse.tile as tile
from concourse import bacc
from concourse.bass_utils import run_bass_kernel_spmd

F32 = mybir.dt.float32
BF16 = mybir.dt.bfloat16
NPBF16 = ml_dtypes.bfloat16
P = 128
HD = 64          # head dim
NHL = 4          # q heads per core
CH = 512         # token chunk (matmul free dim)
NCORES = 8


def build_program(B=2, T=2048, D=2048):
    """Emit the per-core SPMD program. Identical on all cores; inputs differ."""
    BT = B * T
    KT = D // P            # contraction tiles for projections
    NCH = BT // CH         # 512-token chunks over all batches
    NJ = T // CH           # tq chunks per batch
    NI = T // P            # tk tiles per batch
    NTT = BT // P          # token tiles over all batches

    nc = bacc.Bacc(None, target_bir_lowering=False, debug=False)

    xT_d = nc.dram_tensor("xT", [D, BT], BF16, kind="ExternalInput")
    wq_d = nc.dram_tensor("wqT", [D, 256], BF16, kind="ExternalInput")
    wkv_d = nc.dram_tensor("wkvT", [D, 128], BF16, kind="ExternalInput")
    wo_d = nc.dram_tensor("woT", [256, D], BF16, kind="ExternalInput")
    cs_d = nc.dram_tensor("cs", [P, T], BF16, kind="ExternalInput")
    sn_d = nc.dram_tensor("sn", [P, T], BF16, kind="ExternalInput")
    perm_d = nc.dram_tensor("perm", [P, P], BF16, kind="ExternalInput")
    id64_d = nc.dram_tensor("id64", [HD, HD], F32, kind="ExternalInput")
    dmask_d = nc.dram_tensor("dmask", [P, P], F32, kind="ExternalInput")
    out_d = nc.dram_tensor("out", [BT, D], BF16, kind="ExternalOutput")

    with tile.TileContext(nc) as tc:
        with tc.tile_pool(name="persist", bufs=1) as persist:
            wq_sb = persist.tile([P, KT, 256], BF16, tag="wq")
            wkv_sb = persist.tile([P, KT, 128], BF16, tag="wkv")
            wo_sb = persist.tile([P, 2, D], BF16, tag="wo")
            cs_sb = persist.tile([P, T], BF16, tag="cs")
            sn_sb = persist.tile([P, T], BF16, tag="sn")
            perm_sb = persist.tile([P, P], BF16, tag="perm")
            id64_sb = persist.tile([HD, HD], F32, tag="id64")
            dmask_sb = persist.tile([P, P], F32, tag="dmask")
            ones_sb = persist.tile([P, HD], BF16, tag="ones")
            q_sb = persist.tile([P, 2, BT], BF16, tag="qcache")
            k_sb = persist.tile([P, B, T], BF16, tag="kcache")
            v_sb = persist.tile([P, NTT, HD + 1], BF16, tag="vcache")

            nc.sync.dma_start(wq_sb[:], wq_d[:].rearrange("(ko p) m -> p ko m", p=P))
            nc.sync.dma_start(wkv_sb[:], wkv_d[:].rearrange("(ko p) m -> p ko m", p=P))
            nc.sync.dma_start(wo_sb[:], wo_d[:].rearrange("(fo p) n -> p fo n", p=P))
            nc.sync.dma_start(cs_sb[:], cs_d[:])
            nc.sync.dma_start(sn_sb[:], sn_d[:])
            nc.sync.dma_start(perm_sb[:], perm_d[:])
            nc.sync.dma_start(id64_sb[:], id64_d[:])
            nc.sync.dma_start(dmask_sb[:], dmask_d[:])
            nc.vector.memset(v_sb[:, :, HD:HD + 1], 1.0)
            nc.vector.memset(ones_sb[:], 1.0)

            # ---- projections + RoPE (q,k hd-major; v token-major + ones col)
            with (
                tc.tile_pool(name="pa", bufs=5, space="PSUM") as pa,
                tc.tile_pool(name="pb", bufs=2, space="PSUM") as pb,
                tc.tile_pool(name="ptr", bufs=1, space="PSUM") as ptr,
                tc.tile_pool(name="xk", bufs=4) as xkp,
                tc.tile_pool(name="rtmp", bufs=2) as rtmp,
            ):
                for ch in range(NCH):
                    b = ch // NJ
                    tcol = ch * CH               # global token col
                    kcol = CH * (ch % NJ)        # within-batch token col
                    kvp = pa.tile([P, CH], F32, tag="pacc")
                    q0p = pa.tile([P, CH], F32, tag="pacc")
                    q1p = pa.tile([P, CH], F32, tag="pacc")
                    for k in range(KT):
                        xk = xkp.tile([P, CH], BF16, tag="xk")
                        nc.sync.dma_start(
                            xk[:], xT_d[k * P:(k + 1) * P, tcol:tcol + CH])
                        st = (k == 0)
                        sp = (k == KT - 1)
                        nc.tensor.matmul(kvp[:], wkv_sb[:, k, :], xk[:],
                                         start=st, stop=sp)
                        nc.tensor.matmul(q0p[:], wq_sb[:, k, 0:P], xk[:],
                                         start=st, stop=sp)
                        nc.tensor.matmul(q1p[:], wq_sb[:, k, P:256], xk[:],
                                         start=st, stop=sp)
                    csl = cs_sb[:, kcol:kcol + CH]
                    snl = sn_sb[:, kcol:kcol + CH]
                    # q RoPE: rope(q) = q*cos + (P.T@q)*sin
                    for ht, qp in ((0, q0p), (1, q1p)):
                        qs = rtmp.tile([P, CH], BF16, tag="ropea")
                        nc.scalar.copy(qs[:], qp[:])
                        qsw = pb.tile([P, CH], F32, tag="pswap")
                        nc.tensor.matmul(qsw[:], perm_sb[:], qs[:],
                                         start=True, stop=True)
                        dst = q_sb[:, ht, tcol:tcol + CH]
                        nc.vector.tensor_mul(dst, qs[:], csl)
                        t2 = rtmp.tile([P, CH], BF16, tag="ropeb")
                        nc.vector.tensor_mul(t2[:], qsw[:], snl)
                        nc.vector.tensor_add(dst, dst, t2[:])
                    # k RoPE (rows 0:64 of kv psum), then duplicate to 64:128
                    ks = rtmp.tile([HD, CH], BF16, tag="ropek")
                    nc.scalar.copy(ks[:], kvp[0:HD, :])
                    ksw_full = pb.tile([P, CH], F32, tag="pswap")
                    ksw = ksw_full[0:HD, :]
                    nc.tensor.matmul(ksw, perm_sb[0:HD, 0:HD], ks[:],
                                     start=True, stop=True)
                    kdst = k_sb[0:HD, b, kcol:kcol + CH]
                    nc.vector.tensor_mul(kdst, ks[:], cs_sb[0:HD, kcol:kcol + CH])
                    t2 = rtmp.tile([HD, CH], BF16, tag="ropekb")
                    nc.vector.tensor_mul(t2[:], ksw, sn_sb[0:HD, kcol:kcol + CH])
                    nc.vector.tensor_add(kdst, kdst, t2[:])
                    nc.vector.tensor_copy(k_sb[HD:P, b, kcol:kcol + CH], kdst)
                    # v: copy rows 64:128, transpose 128-tok tiles to token-major
                    vs = rtmp.tile([HD, CH], F32, tag="ropev")
                    nc.scalar.copy(vs[:], kvp[HD:P, :])
                    for tt in range(CH // P):
                        vtp = ptr.tile([P, HD], F32, tag="ptr")
                        nc.tensor.transpose(vtp[:], vs[:, tt * P:(tt + 1) * P],
                                            id64_sb[:])
                        nc.vector.tensor_copy(
                            v_sb[:, ch * (CH // P) + tt, 0:HD], vtp[:])

            # ---- attention + inline output projection
            with (
                tc.tile_pool(name="po", bufs=4, space="PSUM") as po,
                tc.tile_pool(name="ps", bufs=2, space="PSUM") as ps,
                tc.tile_pool(name="pout", bufs=2, space="PSUM") as pout,
                tc.tile_pool(name="pp", bufs=4) as pp,
                tc.tile_pool(name="att", bufs=2) as att,
                tc.tile_pool(name="otp", bufs=4) as otp,
            ):
                for b in range(B):
                    for j in range(NJ):
                        ch = b * NJ + j
                        tcol = ch * CH
                        imax = (CH // P) * j + (CH // P) - 1
                        ot_acc = [po.tile([HD + 1, CH], F32, tag="po",
                                          name=f"po_{ch}_{h}")
                                  for h in range(NHL)]
                        for i in range(imax + 1):
                            c0 = max(0, P * i - CH * j)
                            for h in range(NHL):
                                hb = HD * (h % 2)
                                ht = h // 2
                                sp = ps.tile([P, CH], F32, tag="ps")
                                nc.tensor.matmul(
                                    sp[:, c0:CH],
                                    k_sb[hb:hb + HD, b, P * i:P * (i + 1)],
                                    q_sb[hb:hb + HD, ht, tcol + c0:tcol + CH],
                                    start=True, stop=True)
                                if P * i >= CH * j:  # diagonal tile: causal mask
                                    nc.vector.tensor_add(
                                        sp[:, c0:c0 + P], sp[:, c0:c0 + P],
                                        dmask_sb[:])
                                pt = pp.tile([P, CH], BF16, tag="pt")
                                nc.scalar.activation(
                                    pt[:, c0:CH], sp[:, c0:CH],
                                    mybir.ActivationFunctionType.Exp,
                                    scale=0.125)
                                nc.tensor.matmul(
                                    ot_acc[h][:, c0:CH],
                                    v_sb[:, b * NI + i, :],
                                    pt[:, c0:CH],
                                    start=(i == 0), stop=(i == imax),
                                    skip_group_check=True)
                        og_c = att.tile([P, 2, CH], BF16, tag="og")
                        for h in range(NHL):
                            hb = HD * (h % 2)
                            ht = h // 2
                            ot = otp.tile([HD + 1, CH], BF16, tag="ot")
                            nc.scalar.copy(ot[:], ot_acc[h][:])
                            lr_ps = ps.tile([P, CH], F32, tag="ps")
                            nc.tensor.matmul(lr_ps[0:HD, :],
                                             ones_sb[HD:HD + 1, :],
                                             ot[HD:HD + 1, :],
                                             start=True, stop=True)
                            lr = otp.tile([HD, CH], F32, tag="lr")
                            nc.vector.reciprocal(lr[:], lr_ps[0:HD, :])
                            nc.vector.tensor_mul(
                                og_c[hb:hb + HD, ht, :], ot[0:HD, :], lr[:])
                        for tt in range(CH // P):
                            for dc in range(D // CH):
                                op = pout.tile([P, CH], F32, tag="pout")
                                for ft in range(2):
                                    nc.tensor.matmul(
                                        op[:],
                                        og_c[:, ft, tt * P:(tt + 1) * P],
                                        wo_sb[:, ft, dc * CH:(dc + 1) * CH],
                                        start=(ft == 0), stop=(ft == 1))
                                ob = otp.tile([P, CH], BF16, tag="ob")
                                nc.vector.tensor_copy(ob[:], op[:])
                                nc.sync.dma_start(
                                    out_d[tcol + tt * P:tcol + (tt + 1) * P,
                                          dc * CH:(dc + 1) * CH],
                                    ob[:])
    nc.compile()
    return nc


def host_prep(x, wq, wk, wv, wo, cos, sin, core, B=2, T=2048, D=2048):
    """Per-core input map. Core c owns KV head c and Q heads 4c..4c+3."""
    BT = B * T
    xT = np.ascontiguousarray(x.reshape(BT, D).T.astype(NPBF16))
    wqT = np.ascontiguousarray(wq[256 * core:256 * (core + 1)].T.astype(NPBF16))
    wkvT = np.ascontiguousarray(
        np.concatenate([wk[HD * core:HD * (core + 1)],
                        wv[HD * core:HD * (core + 1)]], axis=0).T.astype(NPBF16))
    woT = np.ascontiguousarray(wo[:, 256 * core:256 * (core + 1)].T.astype(NPBF16))
    idx = (np.arange(P) % HD) // 2
    cs = np.ascontiguousarray(cos[:T, idx].T.astype(NPBF16))
    sn = np.ascontiguousarray(sin[:T, idx].T.astype(NPBF16))
    perm = np.zeros((P, P), dtype=NPBF16)
    ii = np.arange(0, P, 2)
    perm[ii, ii + 1] = 1.0
    perm[ii + 1, ii] = -1.0
    dmask = np.where(np.arange(P)[:, None] <= np.arange(P)[None, :],
                     0.0, -1e10).astype(np.float32)
    return {
        "xT": xT, "wqT": wqT, "wkvT": wkvT, "woT": woT,
        "cs": cs, "sn": sn, "perm": perm,
        "id64": np.eye(HD, dtype=np.float32), "dmask": dmask,
    }


_CACHE = {}


def _get_program(B, T, D):
    key = (B, T, D)
    if key not in _CACHE:
        _CACHE[key] = build_program(B, T, D)
    return _CACHE[key]


def run_on_hw(x, wq, wk, wv, wo, cos, sin, B=2, T=2048, D=2048, trace=False, **kw):
    nc = _get_program(B, T, D)
    in_maps = [host_prep(x, wq, wk, wv, wo, cos, sin, c, B, T, D)
               for c in range(NCORES)]
    res = run_bass_kernel_spmd(nc, in_maps, list(range(NCORES)), trace=trace, **kw)
    parts = [np.asarray(r["out"], dtype=np.float32) for r in res.results]
    out = sum(parts).astype(np.float32).reshape(B, T, D)
    return out, res


def kernel(x, mask, wq, wk, wv, wo, cos, sin):
    x = np.asarray(x, dtype=np.float32)
    out, _ = run_on_hw(np.asarray(x, np.float32), np.asarray(wq, np.float32),
                       np.asarray(wk, np.float32), np.asarray(wv, np.float32),
                       np.asarray(wo, np.float32), np.asarray(cos, np.float32),
                       np.asarray(sin, np.float32))
    return out


# revision 6
# speedup vs baseline: 2.4663x; 1.1894x over previous
"""Bass/Tile TRN2 kernel for GroupedQueryAttention (B=2, T=2048, D=2048,
32 Q heads / 8 KV heads, hd=64, RoPE, causal), sharded over 8 NeuronCores
by KV head (1 KV head + 4 Q heads per core; wo row-sharded, partials
summed on host). bf16 datapath (fp32 PSUM accumulation)."""

import sys

for _p in ("/opt/trn_rl_repo",):
    if _p not in sys.path:
        sys.path.insert(0, _p)

import numpy as np
import ml_dtypes

import concourse.bass as bass
import concourse.mybir as mybir
import concourse.tile as tile
from concourse import bacc
from concourse.bass_utils import run_bass_kernel_spmd

F32 = mybir.dt.float32
BF16 = mybir.dt.bfloat16
NPBF16 = ml_dtypes.bfloat16
P = 128
HD = 64          # head dim
NHL = 4          # q heads per core
CH = 512         # token chunk (matmul free dim)
NCORES = 8


def build_program(B=2, T=2048, D=2048):
    """Emit the per-core SPMD program. Identical on all cores; inputs differ."""
    BT = B * T
    KT = D // P            # contraction tiles for projections
    NCH = BT // CH         # 512-token chunks over all batches
    NJ = T // CH           # tq chunks per batch
    NI = T // P            # tk tiles per batch
    NTT = BT // P          # token tiles over all batches

    nc = bacc.Bacc(None, target_bir_lowering=False, debug=False)

    xT_d = nc.dram_tensor("xT", [D, BT], BF16, kind="ExternalInput")
    wq_d = nc.dram_tensor("wqT", [D, 256], BF16, kind="ExternalInput")
    wkv_d = nc.dram_tensor("wkvT", [D, 128], BF16, kind="ExternalInput")
    wo_d = nc.dram_tensor("woT", [256, D], BF16, kind="ExternalInput")
    cs_d = nc.dram_tensor("cs", [P, T], BF16, kind="ExternalInput")
    sn_d = nc.dram_tensor("sn", [P, T], BF16, kind="ExternalInput")
    perm_d = nc.dram_tensor("perm", [P, P], BF16, kind="ExternalInput")
    id64_d = nc.dram_tensor("id64", [HD, HD], F32, kind="ExternalInput")
    dmask_d = nc.dram_tensor("dmask", [P, P], F32, kind="ExternalInput")
    out_d = nc.dram_tensor("out", [BT, D], BF16, kind="ExternalOutput")

    with tile.TileContext(nc) as tc:
        with tc.tile_pool(name="persist", bufs=1) as persist:
            wq_sb = persist.tile([P, KT, 256], BF16, tag="wq")
            wkv_sb = persist.tile([P, KT, 128], BF16, tag="wkv")
            wo_sb = persist.tile([P, 2, D], BF16, tag="wo")
            cs_sb = persist.tile([P, T], BF16, tag="cs")
            sn_sb = persist.tile([P, T], BF16, tag="sn")
            perm_sb = persist.tile([P, P], BF16, tag="perm")
            id64_sb = persist.tile([HD, HD], F32, tag="id64")
            dmask_sb = persist.tile([P, P], F32, tag="dmask")
            ones_sb = persist.tile([P, HD], BF16, tag="ones")
            q_sb = persist.tile([P, 2, BT], BF16, tag="qcache")
            k_sb = persist.tile([P, B, T], BF16, tag="kcache")
            v_sb = persist.tile([P, NTT, HD + 1], BF16, tag="vcache")

            nc.sync.dma_start(wq_sb[:], wq_d[:].rearrange("(ko p) m -> p ko m", p=P))
            nc.sync.dma_start(wkv_sb[:], wkv_d[:].rearrange("(ko p) m -> p ko m", p=P))
            nc.sync.dma_start(wo_sb[:], wo_d[:].rearrange("(fo p) n -> p fo n", p=P))
            nc.sync.dma_start(cs_sb[:], cs_d[:])
            nc.sync.dma_start(sn_sb[:], sn_d[:])
            nc.sync.dma_start(perm_sb[:], perm_d[:])
            nc.sync.dma_start(id64_sb[:], id64_d[:])
            nc.sync.dma_start(dmask_sb[:], dmask_d[:])
            nc.vector.memset(v_sb[:, :, HD:HD + 1], 1.0)
            nc.vector.memset(ones_sb[:], 1.0)

            # ---- projections + RoPE (q,k hd-major; v token-major + ones col)
            with (
                tc.tile_pool(name="pa", bufs=5, space="PSUM") as pa,
                tc.tile_pool(name="pb", bufs=2, space="PSUM") as pb,
                tc.tile_pool(name="ptr", bufs=1, space="PSUM") as ptr,
                tc.tile_pool(name="xk", bufs=4) as xkp,
                tc.tile_pool(name="rtmp", bufs=2) as rtmp,
            ):
                for ch in range(NCH):
                    b = ch // NJ
                    tcol = ch * CH               # global token col
                    kcol = CH * (ch % NJ)        # within-batch token col
                    kvp = pa.tile([P, CH], F32, tag="pacc")
                    q0p = pa.tile([P, CH], F32, tag="pacc")
                    q1p = pa.tile([P, CH], F32, tag="pacc")
                    for k in range(KT):
                        xk = xkp.tile([P, CH], BF16, tag="xk")
                        nc.sync.dma_start(
                            xk[:], xT_d[k * P:(k + 1) * P, tcol:tcol + CH])
                        st = (k == 0)
                        sp = (k == KT - 1)
                        nc.tensor.matmul(kvp[:], wkv_sb[:, k, :], xk[:],
                                         start=st, stop=sp)
                        nc.tensor.matmul(q0p[:], wq_sb[:, k, 0:P], xk[:],
                                         start=st, stop=sp)
                        nc.tensor.matmul(q1p[:], wq_sb[:, k, P:256], xk[:],
                                         start=st, stop=sp)
                    csl = cs_sb[:, kcol:kcol + CH]
                    snl = sn_sb[:, kcol:kcol + CH]
                    # q RoPE: rope(q) = q*cos + (P.T@q)*sin
                    for ht, qp in ((0, q0p), (1, q1p)):
                        qs = rtmp.tile([P, CH], BF16, tag="ropea")
                        nc.scalar.copy(qs[:], qp[:])
                        qsw = pb.tile([P, CH], F32, tag="pswap")
                        nc.tensor.matmul(qsw[:], perm_sb[:], qs[:],
                                         start=True, stop=True)
                        dst = q_sb[:, ht, tcol:tcol + CH]
                        nc.gpsimd.tensor_mul(dst, qs[:], csl)
                        t2 = rtmp.tile([P, CH], BF16, tag="ropeb")
                        nc.vector.tensor_mul(t2[:], qsw[:], snl)
                        nc.vector.tensor_add(dst, dst, t2[:])
                    # k RoPE (rows 0:64 of kv psum), then duplicate to 64:128
                    ks = rtmp.tile([HD, CH], BF16, tag="ropek")
                    nc.scalar.copy(ks[:], kvp[0:HD, :])
                    ksw_full = pb.tile([P, CH], F32, tag="pswap")
                    ksw = ksw_full[0:HD, :]
                    nc.tensor.matmul(ksw, perm_sb[0:HD, 0:HD], ks[:],
                                     start=True, stop=True)
                    kdst = k_sb[0:HD, b, kcol:kcol + CH]
                    nc.vector.tensor_mul(kdst, ks[:], cs_sb[0:HD, kcol:kcol + CH])
                    t2 = rtmp.tile([HD, CH], BF16, tag="ropekb")
                    nc.vector.tensor_mul(t2[:], ksw, sn_sb[0:HD, kcol:kcol + CH])
                    nc.vector.tensor_add(kdst, kdst, t2[:])
                    nc.gpsimd.tensor_copy(k_sb[HD:P, b, kcol:kcol + CH], kdst)
                    # v: copy rows 64:128, transpose 128-tok tiles to token-major
                    vs = rtmp.tile([HD, CH], F32, tag="ropev")
                    nc.scalar.copy(vs[:], kvp[HD:P, :])
                    for tt in range(CH // P):
                        vtp = ptr.tile([P, HD], F32, tag="ptr")
                        nc.tensor.transpose(vtp[:], vs[:, tt * P:(tt + 1) * P],
                                            id64_sb[:])
                        nc.vector.tensor_copy(
                            v_sb[:, ch * (CH // P) + tt, 0:HD], vtp[:])

            # ---- attention + inline output projection
            with (
                tc.tile_pool(name="po", bufs=2, space="PSUM") as po,
                tc.tile_pool(name="ps", bufs=4, space="PSUM") as ps,
                tc.tile_pool(name="pout", bufs=2, space="PSUM") as pout,
                tc.tile_pool(name="pp", bufs=6) as pp,
                tc.tile_pool(name="att", bufs=2) as att,
                tc.tile_pool(name="otp", bufs=4) as otp,
            ):
                for b in range(B):
                    for j in range(NJ):
                        ch = b * NJ + j
                        tcol = ch * CH
                        imax = (CH // P) * j + (CH // P) - 1
                        og_c = att.tile([P, 2, CH], BF16, tag="og")
                        for hp in range(NHL // 2):
                            ot_acc = [po.tile([HD + 1, CH], F32, tag="po",
                                              name=f"po_{ch}_{h}")
                                      for h in (2 * hp, 2 * hp + 1)]
                            for i in range(imax + 1):
                                c0 = max(0, P * i - CH * j)
                                for hi, h in enumerate((2 * hp, 2 * hp + 1)):
                                    hb = HD * (h % 2)
                                    ht = h // 2
                                    sp = ps.tile([P, CH], F32, tag="ps")
                                    nc.tensor.matmul(
                                        sp[:, c0:CH],
                                        k_sb[hb:hb + HD, b, P * i:P * (i + 1)],
                                        q_sb[hb:hb + HD, ht, tcol + c0:tcol + CH],
                                        start=True, stop=True)
                                    if P * i >= CH * j:  # diagonal: causal mask
                                        nc.vector.tensor_add(
                                            sp[:, c0:c0 + P], sp[:, c0:c0 + P],
                                            dmask_sb[:])
                                    pt = pp.tile([P, CH], BF16, tag="pt")
                                    nc.scalar.activation(
                                        pt[:, c0:CH], sp[:, c0:CH],
                                        mybir.ActivationFunctionType.Exp,
                                        scale=0.125)
                                    nc.tensor.matmul(
                                        ot_acc[hi][:, c0:CH],
                                        v_sb[:, b * NI + i, :],
                                        pt[:, c0:CH],
                                        start=(i == 0), stop=(i == imax),
                                        skip_group_check=True)
                            for hi, h in enumerate((2 * hp, 2 * hp + 1)):
                                hb = HD * (h % 2)
                                ht = h // 2
                                # softmax denominator row -> bf16 -> broadcast
                                # via PE -> approx reciprocal -> normalize
                                otr = otp.tile([1, CH], BF16, tag="otr")
                                nc.scalar.copy(
                                    otr[:], ot_acc[hi][HD:HD + 1, :])
                                lr_ps = ps.tile([P, CH], F32, tag="ps")
                                nc.tensor.matmul(lr_ps[0:HD, :],
                                                 ones_sb[0:1, :],
                                                 otr[:],
                                                 start=True, stop=True)
                                lr = otp.tile([HD, CH], F32, tag="lr")
                                nc.vector.reciprocal_approx_fast(
                                    lr[:], lr_ps[0:HD, :])
                                nc.vector.tensor_mul(
                                    og_c[hb:hb + HD, ht, :],
                                    ot_acc[hi][0:HD, :], lr[:])
                        for tt in range(CH // P):
                            for dc in range(D // CH):
                                op = pout.tile([P, CH], F32, tag="pout")
                                for ft in range(2):
                                    nc.tensor.matmul(
                                        op[:],
                                        og_c[:, ft, tt * P:(tt + 1) * P],
                                        wo_sb[:, ft, dc * CH:(dc + 1) * CH],
                                        start=(ft == 0), stop=(ft == 1))
                                ob = otp.tile([P, CH], BF16, tag="ob")
                                nc.vector.tensor_copy(ob[:], op[:])
                                nc.sync.dma_start(
                                    out_d[tcol + tt * P:tcol + (tt + 1) * P,
                                          dc * CH:(dc + 1) * CH],
                                    ob[:])
    nc.compile()
    return nc


def host_prep(x, wq, wk, wv, wo, cos, sin, core, B=2, T=2048, D=2048):
    """Per-core input map. Core c owns KV head c and Q heads 4c..4c+3."""
    BT = B * T
    xT = np.ascontiguousarray(x.reshape(BT, D).T.astype(NPBF16))
    wqT = np.ascontiguousarray(wq[256 * core:256 * (core + 1)].T.astype(NPBF16))
    wkvT = np.ascontiguousarray(
        np.concatenate([wk[HD * core:HD * (core + 1)],
                        wv[HD * core:HD * (core + 1)]], axis=0).T.astype(NPBF16))
    woT = np.ascontiguousarray(wo[:, 256 * core:256 * (core + 1)].T.astype(NPBF16))
    idx = (np.arange(P) % HD) // 2
    cs = np.ascontiguousarray(cos[:T, idx].T.astype(NPBF16))
    sn = np.ascontiguousarray(sin[:T, idx].T.astype(NPBF16))
    perm = np.zeros((P, P), dtype=NPBF16)
    ii = np.arange(0, P, 2)
    perm[ii, ii + 1] = 1.0
    perm[ii + 1, ii] = -1.0
    dmask = np.where(np.arange(P)[:, None] <= np.arange(P)[None, :],
                     0.0, -1e10).astype(np.float32)
    return {
        "xT": xT, "wqT": wqT, "wkvT": wkvT, "woT": woT,
        "cs": cs, "sn": sn, "perm": perm,
        "id64": np.eye(HD, dtype=np.float32), "dmask": dmask,
    }


_CACHE = {}


def _get_program(B, T, D):
    key = (B, T, D)
    if key not in _CACHE:
        _CACHE[key] = build_program(B, T, D)
    return _CACHE[key]


def run_on_hw(x, wq, wk, wv, wo, cos, sin, B=2, T=2048, D=2048, trace=False, **kw):
    nc = _get_program(B, T, D)
    in_maps = [host_prep(x, wq, wk, wv, wo, cos, sin, c, B, T, D)
               for c in range(NCORES)]
    res = run_bass_kernel_spmd(nc, in_maps, list(range(NCORES)), trace=trace, **kw)
    parts = [np.asarray(r["out"], dtype=np.float32) for r in res.results]
    out = sum(parts).astype(np.float32).reshape(B, T, D)
    return out, res


def kernel(x, mask, wq, wk, wv, wo, cos, sin):
    x = np.asarray(x, dtype=np.float32)
    out, _ = run_on_hw(np.asarray(x, np.float32), np.asarray(wq, np.float32),
                       np.asarray(wk, np.float32), np.asarray(wv, np.float32),
                       np.asarray(wo, np.float32), np.asarray(cos, np.float32),
                       np.asarray(sin, np.float32))
    return out


# revision 9
# speedup vs baseline: 2.7900x; 1.1312x over previous
"""Bass/Tile TRN2 kernel for GroupedQueryAttention (B=2, T=2048, D=2048,
32 Q heads / 8 KV heads, hd=64, RoPE, causal), sharded over 8 NeuronCores
by KV head (1 KV head + 4 Q heads per core; wo row-sharded, partials
summed on host). bf16 datapath (fp32 PSUM accumulation)."""

import sys

for _p in ("/opt/trn_rl_repo",):
    if _p not in sys.path:
        sys.path.insert(0, _p)

import numpy as np
import ml_dtypes

import concourse.bass as bass
import concourse.mybir as mybir
import concourse.tile as tile
from concourse import bacc
from concourse.bass_utils import run_bass_kernel_spmd

F32 = mybir.dt.float32
BF16 = mybir.dt.bfloat16
NPBF16 = ml_dtypes.bfloat16
P = 128
HD = 64          # head dim
NHL = 4          # q heads per core
CH = 512         # token chunk (matmul free dim)
NCORES = 8


def build_program(B=2, T=2048, D=2048):
    """Emit the per-core SPMD program. Identical on all cores; inputs differ."""
    BT = B * T
    KT = D // P            # contraction tiles for projections
    NCH = BT // CH         # 512-token chunks over all batches
    NJ = T // CH           # tq chunks per batch
    NI = T // P            # tk tiles per batch
    NTT = BT // P          # token tiles over all batches

    nc = bacc.Bacc(None, target_bir_lowering=False, debug=False)

    xT_d = nc.dram_tensor("xT", [D, BT], BF16, kind="ExternalInput")
    wq_d = nc.dram_tensor("wqT", [D, 256], BF16, kind="ExternalInput")
    wkv_d = nc.dram_tensor("wkvT", [D, 128], BF16, kind="ExternalInput")
    wo_d = nc.dram_tensor("woT", [256, D], BF16, kind="ExternalInput")
    cs_d = nc.dram_tensor("cs", [P, T], BF16, kind="ExternalInput")
    sn_d = nc.dram_tensor("sn", [P, T], BF16, kind="ExternalInput")
    perm_d = nc.dram_tensor("perm", [P, P], BF16, kind="ExternalInput")
    id64_d = nc.dram_tensor("id64", [HD, HD], F32, kind="ExternalInput")
    dmask_d = nc.dram_tensor("dmask", [P, P], F32, kind="ExternalInput")
    out_d = nc.dram_tensor("out", [BT, D], BF16, kind="ExternalOutput")

    with tile.TileContext(nc) as tc:
        with tc.tile_pool(name="persist", bufs=1) as persist:
            wq_sb = persist.tile([P, KT, 256], BF16, tag="wq")
            wkv_sb = persist.tile([P, KT, 128], BF16, tag="wkv")
            wo_sb = persist.tile([P, 2, D], BF16, tag="wo")
            cs_sb = persist.tile([P, T], BF16, tag="cs")
            sn_sb = persist.tile([P, T], BF16, tag="sn")
            perm_sb = persist.tile([P, P], BF16, tag="perm")
            id64_sb = persist.tile([HD, HD], F32, tag="id64")
            dmask_sb = persist.tile([P, P], F32, tag="dmask")
            ones_sb = persist.tile([P, HD], BF16, tag="ones")
            q_sb = persist.tile([P, 2, BT], BF16, tag="qcache")
            k_sb = persist.tile([P, B, T], BF16, tag="kcache")
            v_sb = persist.tile([P, NTT, HD + 1], BF16, tag="vcache")

            nc.sync.dma_start(wq_sb[:], wq_d[:].rearrange("(ko p) m -> p ko m", p=P))
            nc.sync.dma_start(wkv_sb[:], wkv_d[:].rearrange("(ko p) m -> p ko m", p=P))
            nc.sync.dma_start(wo_sb[:], wo_d[:].rearrange("(fo p) n -> p fo n", p=P))
            nc.sync.dma_start(cs_sb[:], cs_d[:])
            nc.sync.dma_start(sn_sb[:], sn_d[:])
            nc.sync.dma_start(perm_sb[:], perm_d[:])
            nc.sync.dma_start(id64_sb[:], id64_d[:])
            nc.sync.dma_start(dmask_sb[:], dmask_d[:])
            nc.vector.memset(v_sb[:, :, HD:HD + 1], 1.0)
            nc.vector.memset(ones_sb[:], 1.0)

            # ---- projections + RoPE (q,k hd-major; v token-major + ones col)
            with (
                tc.tile_pool(name="pa", bufs=5, space="PSUM") as pa,
                tc.tile_pool(name="pb", bufs=2, space="PSUM") as pb,
                tc.tile_pool(name="ptr", bufs=1, space="PSUM") as ptr,
                tc.tile_pool(name="xk", bufs=4) as xkp,
                tc.tile_pool(name="rtmp", bufs=2) as rtmp,
            ):
                for ch in range(NCH):
                    b = ch // NJ
                    tcol = ch * CH               # global token col
                    kcol = CH * (ch % NJ)        # within-batch token col
                    kvp = pa.tile([P, CH], F32, tag="pacc")
                    q0p = pa.tile([P, CH], F32, tag="pacc")
                    q1p = pa.tile([P, CH], F32, tag="pacc")
                    for k in range(KT):
                        xk = xkp.tile([P, CH], BF16, tag="xk")
                        nc.sync.dma_start(
                            xk[:], xT_d[k * P:(k + 1) * P, tcol:tcol + CH])
                        st = (k == 0)
                        sp = (k == KT - 1)
                        nc.tensor.matmul(kvp[:], wkv_sb[:, k, :], xk[:],
                                         start=st, stop=sp)
                        nc.tensor.matmul(q0p[:], wq_sb[:, k, 0:P], xk[:],
                                         start=st, stop=sp)
                        nc.tensor.matmul(q1p[:], wq_sb[:, k, P:256], xk[:],
                                         start=st, stop=sp)
                    csl = cs_sb[:, kcol:kcol + CH]
                    snl = sn_sb[:, kcol:kcol + CH]
                    # q RoPE: rope(q) = q*cos + (P.T@q)*sin
                    for ht, qp in ((0, q0p), (1, q1p)):
                        qs = rtmp.tile([P, CH], BF16, tag="ropea")
                        nc.scalar.copy(qs[:], qp[:])
                        qsw = pb.tile([P, CH], F32, tag="pswap")
                        nc.tensor.matmul(qsw[:], perm_sb[:], qs[:],
                                         start=True, stop=True)
                        dst = q_sb[:, ht, tcol:tcol + CH]
                        nc.gpsimd.tensor_mul(dst, qs[:], csl)
                        t2 = rtmp.tile([P, CH], BF16, tag="ropeb")
                        nc.vector.tensor_mul(t2[:], qsw[:], snl)
                        nc.vector.tensor_add(dst, dst, t2[:])
                    # k RoPE (rows 0:64 of kv psum), then duplicate to 64:128
                    ks = rtmp.tile([HD, CH], BF16, tag="ropek")
                    nc.scalar.copy(ks[:], kvp[0:HD, :])
                    ksw_full = pb.tile([P, CH], F32, tag="pswap")
                    ksw = ksw_full[0:HD, :]
                    nc.tensor.matmul(ksw, perm_sb[0:HD, 0:HD], ks[:],
                                     start=True, stop=True)
                    kdst = k_sb[0:HD, b, kcol:kcol + CH]
                    nc.vector.tensor_mul(kdst, ks[:], cs_sb[0:HD, kcol:kcol + CH])
                    t2 = rtmp.tile([HD, CH], BF16, tag="ropekb")
                    nc.vector.tensor_mul(t2[:], ksw, sn_sb[0:HD, kcol:kcol + CH])
                    nc.vector.tensor_add(kdst, kdst, t2[:])
                    nc.gpsimd.tensor_copy(k_sb[HD:P, b, kcol:kcol + CH], kdst)
                    # v: copy rows 64:128, transpose 128-tok tiles to token-major
                    vs = rtmp.tile([HD, CH], F32, tag="ropev")
                    nc.scalar.copy(vs[:], kvp[HD:P, :])
                    for tt in range(CH // P):
                        vtp = ptr.tile([P, HD], F32, tag="ptr")
                        nc.tensor.transpose(vtp[:], vs[:, tt * P:(tt + 1) * P],
                                            id64_sb[:])
                        nc.vector.tensor_copy(
                            v_sb[:, ch * (CH // P) + tt, 0:HD], vtp[:])

            # ---- attention + software-pipelined output projection
            # Out-proj matmuls of chunk N-1 are emitted INSIDE chunk N's
            # attention i-loop: the PE FIFO is in-order, and the exp
            # (ACT) rate-limits attV, so these fill the PE stalls that
            # otherwise trigger HAM downclocking.
            with (
                tc.tile_pool(name="po", bufs=2, space="PSUM") as po,
                tc.tile_pool(name="ps", bufs=4, space="PSUM") as ps,
                tc.tile_pool(name="pout", bufs=2, space="PSUM") as pout,
                tc.tile_pool(name="pp", bufs=6) as pp,
                tc.tile_pool(name="att", bufs=2) as att,
                tc.tile_pool(name="otp", bufs=6) as otp,
            ):
                pending = []  # deferred out-proj units: (og_c, tcol, tt, dc)

                def emit_outproj(og_p, tcol_p, tt, dc):
                    op = pout.tile([P, CH], F32, tag="pout")
                    for ft in range(2):
                        nc.tensor.matmul(
                            op[:],
                            og_p[:, ft, tt * P:(tt + 1) * P],
                            wo_sb[:, ft, dc * CH:(dc + 1) * CH],
                            start=(ft == 0), stop=(ft == 1))
                    ob = otp.tile([P, CH], BF16, tag="ob")
                    nc.vector.tensor_copy(ob[:], op[:])
                    nc.sync.dma_start(
                        out_d[tcol_p + tt * P:tcol_p + (tt + 1) * P,
                              dc * CH:(dc + 1) * CH],
                        ob[:])

                for b in range(B):
                    for j in range(NJ):
                        ch = b * NJ + j
                        tcol = ch * CH
                        imax = (CH // P) * j + (CH // P) - 1
                        og_c = att.tile([P, 2, CH], BF16, tag="og")
                        for hp in range(NHL // 2):
                            ot_acc = [po.tile([HD + 1, CH], F32, tag="po",
                                              name=f"po_{ch}_{h}")
                                      for h in (2 * hp, 2 * hp + 1)]
                            for i in range(imax + 1):
                                c0 = max(0, P * i - CH * j)
                                for hi, h in enumerate((2 * hp, 2 * hp + 1)):
                                    hb = HD * (h % 2)
                                    ht = h // 2
                                    sp = ps.tile([P, CH], F32, tag="ps")
                                    nc.tensor.matmul(
                                        sp[:, c0:CH],
                                        k_sb[hb:hb + HD, b, P * i:P * (i + 1)],
                                        q_sb[hb:hb + HD, ht, tcol + c0:tcol + CH],
                                        start=True, stop=True)
                                    if P * i >= CH * j:  # diagonal: causal mask
                                        nc.vector.tensor_add(
                                            sp[:, c0:c0 + P], sp[:, c0:c0 + P],
                                            dmask_sb[:])
                                    pt = pp.tile([P, CH], BF16, tag="pt")
                                    nc.scalar.activation(
                                        pt[:, c0:CH], sp[:, c0:CH],
                                        mybir.ActivationFunctionType.Exp,
                                        scale=0.125)
                                    nc.tensor.matmul(
                                        ot_acc[hi][:, c0:CH],
                                        v_sb[:, b * NI + i, :],
                                        pt[:, c0:CH],
                                        start=(i == 0), stop=(i == imax),
                                        skip_group_check=True)
                                if pending:
                                    emit_outproj(*pending.pop(0))
                            for hi, h in enumerate((2 * hp, 2 * hp + 1)):
                                hb = HD * (h % 2)
                                ht = h // 2
                                # softmax denominator row -> bf16 -> broadcast
                                # via PE -> approx reciprocal -> normalize
                                otr = otp.tile([1, CH], BF16, tag="otr")
                                nc.scalar.copy(
                                    otr[:], ot_acc[hi][HD:HD + 1, :])
                                lr_ps = ps.tile([P, CH], F32, tag="ps")
                                nc.tensor.matmul(lr_ps[0:HD, :],
                                                 ones_sb[0:1, :],
                                                 otr[:],
                                                 start=True, stop=True)
                                lr = otp.tile([HD, CH], F32, tag="lr")
                                nc.vector.reciprocal_approx_fast(
                                    lr[:], lr_ps[0:HD, :])
                                nc.vector.tensor_mul(
                                    og_c[hb:hb + HD, ht, :],
                                    ot_acc[hi][0:HD, :], lr[:])
                            for _ in range(4):
                                if pending:
                                    emit_outproj(*pending.pop(0))
                        pending.extend(
                            (og_c, tcol, tt, dc)
                            for tt in range(CH // P) for dc in range(D // CH))
                while pending:
                    emit_outproj(*pending.pop(0))
    nc.compile()
    return nc


def host_prep(x, wq, wk, wv, wo, cos, sin, core, B=2, T=2048, D=2048):
    """Per-core input map. Core c owns KV head c and Q heads 4c..4c+3."""
    BT = B * T
    xT = np.ascontiguousarray(x.reshape(BT, D).T.astype(NPBF16))
    wqT = np.ascontiguousarray(wq[256 * core:256 * (core + 1)].T.astype(NPBF16))
    wkvT = np.ascontiguousarray(
        np.concatenate([wk[HD * core:HD * (core + 1)],
                        wv[HD * core:HD * (core + 1)]], axis=0).T.astype(NPBF16))
    woT = np.ascontiguousarray(wo[:, 256 * core:256 * (core + 1)].T.astype(NPBF16))
    idx = (np.arange(P) % HD) // 2
    cs = np.ascontiguousarray(cos[:T, idx].T.astype(NPBF16))
    sn = np.ascontiguousarray(sin[:T, idx].T.astype(NPBF16))
    perm = np.zeros((P, P), dtype=NPBF16)
    ii = np.arange(0, P, 2)
    perm[ii, ii + 1] = 1.0
    perm[ii + 1, ii] = -1.0
    dmask = np.where(np.arange(P)[:, None] <= np.arange(P)[None, :],
                     0.0, -1e10).astype(np.float32)
    return {
        "xT": xT, "wqT": wqT, "wkvT": wkvT, "woT": woT,
        "cs": cs, "sn": sn, "perm": perm,
        "id64": np.eye(HD, dtype=np.float32), "dmask": dmask,
    }


_CACHE = {}


def _get_program(B, T, D):
    key = (B, T, D)
    if key not in _CACHE:
        _CACHE[key] = build_program(B, T, D)
    return _CACHE[key]


def run_on_hw(x, wq, wk, wv, wo, cos, sin, B=2, T=2048, D=2048, trace=False, **kw):
    nc = _get_program(B, T, D)
    in_maps = [host_prep(x, wq, wk, wv, wo, cos, sin, c, B, T, D)
               for c in range(NCORES)]
    res = run_bass_kernel_spmd(nc, in_maps, list(range(NCORES)), trace=trace, **kw)
    parts = [np.asarray(r["out"], dtype=np.float32) for r in res.results]
    out = sum(parts).astype(np.float32).reshape(B, T, D)
    return out, res


def kernel(x, mask, wq, wk, wv, wo, cos, sin):
    x = np.asarray(x, dtype=np.float32)
    out, _ = run_on_hw(np.asarray(x, np.float32), np.asarray(wq, np.float32),
                       np.asarray(wk, np.float32), np.asarray(wv, np.float32),
                       np.asarray(wo, np.float32), np.asarray(cos, np.float32),
                       np.asarray(sin, np.float32))
    return out


# revision 11
# speedup vs baseline: 2.9634x; 1.0622x over previous
"""Bass/Tile TRN2 kernel for GroupedQueryAttention (B=2, T=2048, D=2048,
32 Q heads / 8 KV heads, hd=64, RoPE, causal), sharded over 8 NeuronCores
by KV head (1 KV head + 4 Q heads per core; wo row-sharded, partials
summed on host). bf16 datapath (fp32 PSUM accumulation)."""

import sys

for _p in ("/opt/trn_rl_repo",):
    if _p not in sys.path:
        sys.path.insert(0, _p)

import numpy as np
import ml_dtypes

import concourse.bass as bass
import concourse.mybir as mybir
import concourse.tile as tile
from concourse import bacc
from concourse.bass_utils import run_bass_kernel_spmd

F32 = mybir.dt.float32
BF16 = mybir.dt.bfloat16
NPBF16 = ml_dtypes.bfloat16
P = 128
HD = 64          # head dim
NHL = 4          # q heads per core
CH = 512         # token chunk (matmul free dim)
NCORES = 8


def build_program(B=2, T=2048, D=2048):
    """Emit the per-core SPMD program. Identical on all cores; inputs differ."""
    BT = B * T
    KT = D // P            # contraction tiles for projections
    NCH = BT // CH         # 512-token chunks over all batches
    NJ = T // CH           # tq chunks per batch
    NI = T // P            # tk tiles per batch
    NTT = BT // P          # token tiles over all batches

    nc = bacc.Bacc(None, target_bir_lowering=False, debug=False)

    xT_d = nc.dram_tensor("xT", [D, BT], BF16, kind="ExternalInput")
    wq_d = nc.dram_tensor("wqT", [D, 256], BF16, kind="ExternalInput")
    wkv_d = nc.dram_tensor("wkvT", [D, 128], BF16, kind="ExternalInput")
    wo_d = nc.dram_tensor("woT", [256, D], BF16, kind="ExternalInput")
    cs_d = nc.dram_tensor("cs", [P, T], BF16, kind="ExternalInput")
    sn_d = nc.dram_tensor("sn", [P, T], BF16, kind="ExternalInput")
    perm_d = nc.dram_tensor("perm", [P, P], BF16, kind="ExternalInput")
    id64_d = nc.dram_tensor("id64", [HD, HD], F32, kind="ExternalInput")
    dmask_d = nc.dram_tensor("dmask", [P, P], F32, kind="ExternalInput")
    out_d = nc.dram_tensor("out", [BT, D], BF16, kind="ExternalOutput")

    with tile.TileContext(nc) as tc:
        with tc.tile_pool(name="persist", bufs=1) as persist:
            wq_sb = persist.tile([P, KT, 256], BF16, tag="wq")
            wkv_sb = persist.tile([P, KT, 128], BF16, tag="wkv")
            wo_sb = persist.tile([P, 2, D], BF16, tag="wo")
            cs_sb = persist.tile([P, T], BF16, tag="cs")
            sn_sb = persist.tile([P, T], BF16, tag="sn")
            perm_sb = persist.tile([P, P], BF16, tag="perm")
            id64_sb = persist.tile([HD, HD], F32, tag="id64")
            dmask_sb = persist.tile([P, 2, P], F32, tag="dmask")
            ones_sb = persist.tile([P, HD], BF16, tag="ones")
            q_sb = persist.tile([P, 2, BT], BF16, tag="qcache")
            k_sb = persist.tile([P, B, T], BF16, tag="kcache")
            v_sb = persist.tile([P, NTT, HD + 1], BF16, tag="vcache")

            nc.sync.dma_start(wq_sb[:], wq_d[:].rearrange("(ko p) m -> p ko m", p=P))
            nc.sync.dma_start(wkv_sb[:], wkv_d[:].rearrange("(ko p) m -> p ko m", p=P))
            nc.sync.dma_start(wo_sb[:], wo_d[:].rearrange("(fo p) n -> p fo n", p=P))
            nc.sync.dma_start(cs_sb[:], cs_d[:])
            nc.sync.dma_start(sn_sb[:], sn_d[:])
            nc.sync.dma_start(perm_sb[:], perm_d[:])
            nc.sync.dma_start(id64_sb[:], id64_d[:])
            nc.sync.dma_start(dmask_sb[:, 0, :], dmask_d[:])
            nc.sync.dma_start(dmask_sb[:, 1, :], dmask_d[:])
            nc.vector.memset(v_sb[:, :, HD:HD + 1], 1.0)
            nc.vector.memset(ones_sb[:], 1.0)

            # ---- projections + RoPE (q,k hd-major; v token-major + ones col)
            with (
                tc.tile_pool(name="pa", bufs=5, space="PSUM") as pa,
                tc.tile_pool(name="pb", bufs=2, space="PSUM") as pb,
                tc.tile_pool(name="ptr", bufs=1, space="PSUM") as ptr,
                tc.tile_pool(name="xk", bufs=4) as xkp,
                tc.tile_pool(name="rtmp", bufs=2) as rtmp,
            ):
                for ch in range(NCH):
                    b = ch // NJ
                    tcol = ch * CH               # global token col
                    kcol = CH * (ch % NJ)        # within-batch token col
                    kvp = pa.tile([P, CH], F32, tag="pacc")
                    q0p = pa.tile([P, CH], F32, tag="pacc")
                    q1p = pa.tile([P, CH], F32, tag="pacc")
                    for k in range(KT):
                        xk = xkp.tile([P, CH], BF16, tag="xk")
                        nc.sync.dma_start(
                            xk[:], xT_d[k * P:(k + 1) * P, tcol:tcol + CH])
                        st = (k == 0)
                        sp = (k == KT - 1)
                        nc.tensor.matmul(kvp[:], wkv_sb[:, k, :], xk[:],
                                         start=st, stop=sp)
                        nc.tensor.matmul(q0p[:], wq_sb[:, k, 0:P], xk[:],
                                         start=st, stop=sp)
                        nc.tensor.matmul(q1p[:], wq_sb[:, k, P:256], xk[:],
                                         start=st, stop=sp)
                    csl = cs_sb[:, kcol:kcol + CH]
                    snl = sn_sb[:, kcol:kcol + CH]
                    # q RoPE: rope(q) = q*cos + (P.T@q)*sin
                    for ht, qp in ((0, q0p), (1, q1p)):
                        qs = rtmp.tile([P, CH], BF16, tag="ropea")
                        nc.scalar.copy(qs[:], qp[:])
                        qsw = pb.tile([P, CH], F32, tag="pswap")
                        nc.tensor.matmul(qsw[:], perm_sb[:], qs[:],
                                         start=True, stop=True)
                        dst = q_sb[:, ht, tcol:tcol + CH]
                        nc.gpsimd.tensor_mul(dst, qs[:], csl)
                        t2 = rtmp.tile([P, CH], BF16, tag="ropeb")
                        nc.vector.tensor_mul(t2[:], qsw[:], snl)
                        nc.vector.tensor_add(dst, dst, t2[:])
                    # k RoPE (rows 0:64 of kv psum), then duplicate to 64:128
                    ks = rtmp.tile([HD, CH], BF16, tag="ropek")
                    nc.scalar.copy(ks[:], kvp[0:HD, :])
                    ksw_full = pb.tile([P, CH], F32, tag="pswap")
                    ksw = ksw_full[0:HD, :]
                    nc.tensor.matmul(ksw, perm_sb[0:HD, 0:HD], ks[:],
                                     start=True, stop=True)
                    kdst = k_sb[0:HD, b, kcol:kcol + CH]
                    nc.vector.tensor_mul(kdst, ks[:], cs_sb[0:HD, kcol:kcol + CH])
                    t2 = rtmp.tile([HD, CH], BF16, tag="ropekb")
                    nc.vector.tensor_mul(t2[:], ksw, sn_sb[0:HD, kcol:kcol + CH])
                    nc.vector.tensor_add(kdst, kdst, t2[:])
                    nc.gpsimd.tensor_copy(k_sb[HD:P, b, kcol:kcol + CH], kdst)
                    # v: copy rows 64:128, transpose 128-tok tiles to token-major
                    vs = rtmp.tile([HD, CH], F32, tag="ropev")
                    nc.scalar.copy(vs[:], kvp[HD:P, :])
                    for tt in range(CH // P):
                        vtp = ptr.tile([P, HD], F32, tag="ptr")
                        nc.tensor.transpose(vtp[:], vs[:, tt * P:(tt + 1) * P],
                                            id64_sb[:])
                        nc.vector.tensor_copy(
                            v_sb[:, ch * (CH // P) + tt, 0:HD], vtp[:])

            # ---- attention + software-pipelined output projection
            # Out-proj matmuls of chunk N-1 are emitted INSIDE chunk N's
            # attention i-loop: the PE FIFO is in-order, and the exp
            # (ACT) rate-limits attV, so these fill the PE stalls that
            # otherwise trigger HAM downclocking.
            with (
                tc.tile_pool(name="po", bufs=2, space="PSUM") as po,
                tc.tile_pool(name="ps", bufs=2, space="PSUM") as ps,
                tc.tile_pool(name="pout", bufs=2, space="PSUM") as pout,
                tc.tile_pool(name="pp", bufs=6) as pp,
                tc.tile_pool(name="att", bufs=2) as att,
                tc.tile_pool(name="otp", bufs=6) as otp,
            ):
                pending = []  # deferred out-proj units: (og_c, tcol, tt, dc)

                def emit_outproj(og_p, tcol_p, tt, dc):
                    op = pout.tile([P, CH], F32, tag="pout")
                    for ft in range(2):
                        nc.tensor.matmul(
                            op[:],
                            og_p[:, ft, tt * P:(tt + 1) * P],
                            wo_sb[:, ft, dc * CH:(dc + 1) * CH],
                            start=(ft == 0), stop=(ft == 1))
                    ob = otp.tile([P, CH], BF16, tag="ob")
                    nc.vector.tensor_copy(ob[:], op[:])
                    nc.sync.dma_start(
                        out_d[tcol_p + tt * P:tcol_p + (tt + 1) * P,
                              dc * CH:(dc + 1) * CH],
                        ob[:])

                for b in range(B):
                    for j in range(NJ):
                        ch = b * NJ + j
                        tcol = ch * CH
                        imax = (CH // P) * j + (CH // P) - 1
                        og_c = att.tile([P, 2, CH], BF16, tag="og")
                        for hp in range(NHL // 2):
                            ot_acc = [po.tile([HD + 1, CH], F32, tag="po",
                                              name=f"po_{ch}_{h}")
                                      for h in (2 * hp, 2 * hp + 1)]
                            for i in range(imax + 1):
                                c0 = max(0, P * i - CH * j)
                                sp2 = ps.tile([P, 2, CH], F32, tag="ps")
                                for hi, h in enumerate((2 * hp, 2 * hp + 1)):
                                    hb = HD * (h % 2)
                                    ht = h // 2
                                    nc.tensor.matmul(
                                        sp2[:, hi, c0:CH],
                                        k_sb[hb:hb + HD, b, P * i:P * (i + 1)],
                                        q_sb[hb:hb + HD, ht, tcol + c0:tcol + CH],
                                        start=True, stop=True)
                                if P * i >= CH * j:  # diagonal: causal mask
                                    nc.vector.tensor_add(
                                        sp2[:, :, c0:c0 + P],
                                        sp2[:, :, c0:c0 + P],
                                        dmask_sb[:])
                                pt2 = pp.tile([P, 2, CH], BF16, tag="pt")
                                nc.scalar.activation(
                                    pt2[:, :, c0:CH], sp2[:, :, c0:CH],
                                    mybir.ActivationFunctionType.Exp,
                                    scale=0.125)
                                for hi in range(2):
                                    nc.tensor.matmul(
                                        ot_acc[hi][:, c0:CH],
                                        v_sb[:, b * NI + i, :],
                                        pt2[:, hi, c0:CH],
                                        start=(i == 0), stop=(i == imax),
                                        skip_group_check=True)
                                if pending:
                                    emit_outproj(*pending.pop(0))
                            for hi, h in enumerate((2 * hp, 2 * hp + 1)):
                                hb = HD * (h % 2)
                                ht = h // 2
                                # softmax denominator row -> SBUF partition 0
                                # (recip_approx misreads PSUM at partition
                                # offset 64), approx-reciprocal, broadcast
                                # across partitions on GpSimd
                                otr = otp.tile([1, CH], F32, tag="otr")
                                nc.vector.tensor_copy(
                                    otr[:], ot_acc[hi][HD:HD + 1, :])
                                dr = otp.tile([1, CH], F32, tag="dr")
                                nc.vector.reciprocal_approx_fast(
                                    dr[:], otr[:])
                                lr = otp.tile([HD, CH], F32, tag="lr")
                                nc.gpsimd.partition_broadcast(lr[:], dr[:])
                                nc.vector.tensor_mul(
                                    og_c[hb:hb + HD, ht, :],
                                    ot_acc[hi][0:HD, :], lr[:])
                            for _ in range(4):
                                if pending:
                                    emit_outproj(*pending.pop(0))
                        pending.extend(
                            (og_c, tcol, tt, dc)
                            for tt in range(CH // P) for dc in range(D // CH))
                while pending:
                    emit_outproj(*pending.pop(0))
    nc.compile()
    return nc


def host_prep(x, wq, wk, wv, wo, cos, sin, core, B=2, T=2048, D=2048):
    """Per-core input map. Core c owns KV head c and Q heads 4c..4c+3."""
    BT = B * T
    xT = np.ascontiguousarray(x.reshape(BT, D).T.astype(NPBF16))
    wqT = np.ascontiguousarray(wq[256 * core:256 * (core + 1)].T.astype(NPBF16))
    wkvT = np.ascontiguousarray(
        np.concatenate([wk[HD * core:HD * (core + 1)],
                        wv[HD * core:HD * (core + 1)]], axis=0).T.astype(NPBF16))
    woT = np.ascontiguousarray(wo[:, 256 * core:256 * (core + 1)].T.astype(NPBF16))
    idx = (np.arange(P) % HD) // 2
    cs = np.ascontiguousarray(cos[:T, idx].T.astype(NPBF16))
    sn = np.ascontiguousarray(sin[:T, idx].T.astype(NPBF16))
    perm = np.zeros((P, P), dtype=NPBF16)
    ii = np.arange(0, P, 2)
    perm[ii, ii + 1] = 1.0
    perm[ii + 1, ii] = -1.0
    dmask = np.where(np.arange(P)[:, None] <= np.arange(P)[None, :],
                     0.0, -1e10).astype(np.float32)
    return {
        "xT": xT, "wqT": wqT, "wkvT": wkvT, "woT": woT,
        "cs": cs, "sn": sn, "perm": perm,
        "id64": np.eye(HD, dtype=np.float32), "dmask": dmask,
    }


_CACHE = {}


def _get_program(B, T, D):
    key = (B, T, D)
    if key not in _CACHE:
        _CACHE[key] = build_program(B, T, D)
    return _CACHE[key]


def run_on_hw(x, wq, wk, wv, wo, cos, sin, B=2, T=2048, D=2048, trace=False, **kw):
    nc = _get_program(B, T, D)
    in_maps = [host_prep(x, wq, wk, wv, wo, cos, sin, c, B, T, D)
               for c in range(NCORES)]
    res = run_bass_kernel_spmd(nc, in_maps, list(range(NCORES)), trace=trace, **kw)
    parts = [np.asarray(r["out"], dtype=np.float32) for r in res.results]
    out = sum(parts).astype(np.float32).reshape(B, T, D)
    return out, res


def kernel(x, mask, wq, wk, wv, wo, cos, sin):
    x = np.asarray(x, dtype=np.float32)
    out, _ = run_on_hw(np.asarray(x, np.float32), np.asarray(wq, np.float32),
                       np.asarray(wk, np.float32), np.asarray(wv, np.float32),
                       np.asarray(wo, np.float32), np.asarray(cos, np.float32),
                       np.asarray(sin, np.float32))
    return out
